# revision 20
# baseline (speedup 1.0000x reference)
"""Trainium2 Bass kernel for nn_MultiHeadAttention_87239375716860.

Softmax-over-HEADS MHA (reference quirk). Sharding: 8 cores = 4 batches
x 2 query-halves, no collectives.

v2 changes vs baseline:
  - all matmul operands bf16 (host converts); input DMA halved
  - attn@V in o[q,e] form: lhsT = a_h[k,128q], rhs = v[k,64e] -> full
    128-contraction PE efficiency (halves that phase's PE columns)
  - out-projection via PE transposes of o[q,e] (identity input tensor)
  - softmax engine rebalance: exp on Act, tree l1/l3/l4+recip+muls on
    DVE, l2 on Pool, v/y biases on Pool
  - k/v projections pipelined inside qb0's attention sweep
PSUM budget: psum_s tag "s" [128,1024] x2 bufs (4 banks) shared by
scores / transposes / y; psum_o tags o0,o1 [128,1024] x1 buf (4 banks)
for the two q-half o accumulators.  start/stop are BANK-granular.
"""

import numpy as np

SEQ = 2048
BATCH = 4
D = 1024
H = 16
DK = 64
QCH = 1024          # queries per core
NCORES = 8
QB = 256            # q-block for scores/softmax
NQB = QCH // QB     # 4
NKT = SEQ // 128    # 16 k-tiles

_CACHE = {}


def _build_bass(n_iters=1):
    import concourse.tile as tile
    from concourse import bacc, mybir

    f32 = mybir.dt.float32
    bf16 = mybir.dt.bfloat16

    nc = bacc.Bacc("TRN2", target_bir_lowering=False, debug=False,
                   num_devices=NCORES)

    d = {}
    d["xqT"] = nc.dram_tensor("xqT", [D, QCH], bf16, kind="ExternalInput").ap()
    d["xkT"] = nc.dram_tensor("xkT", [D, SEQ], bf16, kind="ExternalInput").ap()
    d["xvT"] = nc.dram_tensor("xvT", [D, SEQ], bf16, kind="ExternalInput").ap()
    d["wsT"] = nc.dram_tensor("wsT", [D, D], bf16, kind="ExternalInput").ap()
    d["woT"] = nc.dram_tensor("woT", [D, D], bf16, kind="ExternalInput").ap()
    d["bq"] = nc.dram_tensor("bq", [128, 8], f32, kind="ExternalInput").ap()
    d["bk"] = nc.dram_tensor("bk", [128, 8], f32, kind="ExternalInput").ap()
    d["bvro"] = nc.dram_tensor("bvro", [1, D], bf16,
                               kind="ExternalInput").ap()
    d["boro"] = nc.dram_tensor("boro", [1, D], bf16,
                               kind="ExternalInput").ap()
    d["ident"] = nc.dram_tensor("ident", [128, 128], bf16,
                                kind="ExternalInput").ap()
    d["out"] = nc.dram_tensor("out", [QCH, D], f32, kind="ExternalOutput").ap()

    with tile.TileContext(nc) as tc:
        for _it in range(n_iters):
            _emit_iter(nc, tc, mybir, tile, d)

    nc.compile()
    return nc


def _emit_iter(nc, tc, mybir, tile, d):
    from contextlib import ExitStack

    f32 = mybir.dt.float32
    bf16 = mybir.dt.bfloat16
    AF = mybir.ActivationFunctionType

    with ExitStack() as ctx:
        persist = ctx.enter_context(tc.tile_pool(name="persist", bufs=1))
        psum_s = ctx.enter_context(
            tc.tile_pool(name="psum_s", space="PSUM", bufs=2))
        psum_o = ctx.enter_context(
            tc.tile_pool(name="psum_o", space="PSUM", bufs=1))
        work = ctx.enter_context(tc.tile_pool(name="work", bufs=1))

        # wio holds Ws^T during projections, then Wo^T (loaded after the
        # last vproj) for the out-projection.
        wio = persist.tile([128, 8, D], bf16, name="wio")
        qT2 = persist.tile([128, 8, 2, QCH], bf16, name="qT2")
        kT = persist.tile([128, 8, SEQ], bf16, name="kT")
        v = persist.tile([128, NKT, H, DK], bf16, name="v")
        id_t = persist.tile([128, 128], bf16, name="id_t")
        bq_t = persist.tile([128, 8], f32, name="bq_t")
        bk_t = persist.tile([128, 8], f32, name="bk_t")
        bvro_t = persist.tile([1, D], bf16, name="bvro_t")
        boro_t = persist.tile([1, D], bf16, name="boro_t")
        ones_t = persist.tile([1, 128], bf16, name="ones_t")

        nc.sync.dma_start(wio[:], d["wsT"].rearrange("(c p) e -> p c e",
                                                     p=128))
        nc.gpsimd.memset(qT2[64:128, :, 0, :], 0.0)
        nc.gpsimd.memset(qT2[0:64, :, 1, :], 0.0)
        nc.sync.dma_start(id_t[:], d["ident"])
        nc.sync.dma_start(bq_t[:], d["bq"])
        nc.sync.dma_start(bk_t[:], d["bk"])
        nc.sync.dma_start(bvro_t[:], d["bvro"])
        nc.sync.dma_start(boro_t[:], d["boro"])
        nc.gpsimd.memset(ones_t[:], 1.0)

        # ---- qT projection (zero-padded parity trick) ----
        for sb in range(QCH // 512):
            xq_s = work.tile([128, 8, 512], bf16, tag="xs", bufs=2,
                             name=f"xq{sb}")
            nc.sync.dma_start(
                xq_s[:], d["xqT"][:, sb * 512:(sb + 1) * 512]
                .rearrange("(c p) s -> p c s", p=128))
            for c in range(8):
                ps = psum_s.tile([128, 1024], f32, tag="s", name=f"psq{sb}{c}")
                for dch in range(8):
                    nc.tensor.matmul(ps[:, 0:512],
                                     wio[:, dch, c * 128:(c + 1) * 128],
                                     xq_s[:, dch, :],
                                     start=(dch == 0), stop=(dch == 7))
                nc.scalar.activation(
                    qT2[0:64, c, 0, sb * 512:(sb + 1) * 512], ps[0:64, 0:512],
                    AF.Identity, bias=bq_t[0:64, c:c + 1], scale=0.125)
                nc.scalar.activation(
                    qT2[64:128, c, 1, sb * 512:(sb + 1) * 512],
                    ps[64:128, 0:512],
                    AF.Identity, bias=bq_t[64:128, c:c + 1], scale=0.125)

        def kproj(sb):
            xk_s = work.tile([128, 8, 512], bf16, tag="xs", bufs=2,
                             name=f"xk{sb}")
            nc.sync.dma_start(
                xk_s[:], d["xkT"][:, sb * 512:(sb + 1) * 512]
                .rearrange("(c p) s -> p c s", p=128))
            for c in range(8):
                ps = psum_s.tile([128, 1024], f32, tag="s", name=f"psk{sb}{c}")
                for dch in range(8):
                    nc.tensor.matmul(ps[:, 0:512],
                                     wio[:, dch, c * 128:(c + 1) * 128],
                                     xk_s[:, dch, :],
                                     start=(dch == 0), stop=(dch == 7))
                nc.scalar.activation(kT[:, c, sb * 512:(sb + 1) * 512],
                                     ps[:, 0:512], AF.Identity,
                                     bias=bk_t[:, c:c + 1])

        def vproj(kch):
            xv_s = work.tile([128, 8, 128], bf16, tag="xv", bufs=2,
                             name=f"xv{kch}")
            nc.sync.dma_start(
                xv_s[:], d["xvT"][:, kch * 128:(kch + 1) * 128]
                .rearrange("(c p) k -> p c k", p=128))
            ps = psum_s.tile([128, 1024], f32, tag="s", name=f"psv{kch}")
            for eb in range(2):
                for dch in range(8):
                    nc.tensor.matmul(ps[:, eb * 512:(eb + 1) * 512],
                                     xv_s[:, dch, :],
                                     wio[:, dch, eb * 512:(eb + 1) * 512],
                                     start=(dch == 0), stop=False)
                # bias: ones^T (x) b_split row, contraction-1 accumulate
                nc.tensor.matmul(ps[:, eb * 512:(eb + 1) * 512], ones_t[:],
                                 bvro_t[0:1, eb * 512:(eb + 1) * 512],
                                 start=False, stop=True)
            pv = ps.rearrange("p (h k) -> p h k", h=H)
            nc.scalar.activation(v[:, kch, :, :], pv[:], AF.Copy)

        # ---- software-pipelined attention ----
        # stage layout per emission step (qb, kt):
        #   PE : scores(kt) hg0,hg1 | av(kt-2) h0-7 | scores hg2,hg3 |
        #        av(kt-2) h8-15
        #   Act: exp(kt) hg0..3
        #   Pool: l1(kt)  (after exp hg3)
        #   DVE: l2..muls of (kt-1)
        es = {}          # (qb,kt) -> e tile
        o_ps_all = {}

        def scores_half(qb, kt, half, e):
            q0 = qb * QB
            for hg in (0, 1) if half == 0 else (2, 3):
                ps = psum_s.tile([128, 1024], f32, tag="s",
                                 name=f"pss{qb}_{kt}_{hg}")
                for cl in range(2):
                    c = hg * 2 + cl
                    nc.tensor.matmul(ps[:, cl * 512:(cl + 1) * 512],
                                     kT[:, c, kt * 128:(kt + 1) * 128],
                                     qT2[:, c, :, q0:q0 + QB],
                                     start=True, stop=True)
                nc.scalar.activation(e[:, hg * 4:(hg + 1) * 4, :], ps[:, :],
                                     AF.Exp)

        def tree_l1a(qb, kt):
            e = es[(qb, kt)]
            u = work.tile([128, 2, 4, QB], bf16, tag="u", bufs=2,
                          name=f"u{qb}_{kt}")
            nc.vector.tensor_add(u[:, 0, :, :], e[:, 0:4, :], e[:, 4:8, :])
            return u

        def tree_l1b(qb, kt):
            e, u = es[(qb, kt)], us[(qb, kt)]
            nc.vector.tensor_add(u[:, 1, :, :], e[:, 8:12, :], e[:, 12:16, :])

        us = {}

        u2s = {}

        def l2_piece(qb, kt):
            u = us[(qb, kt)]
            u2 = work.tile([128, 4, QB], bf16, tag="u2", bufs=2,
                           name=f"u2_{qb}_{kt}")
            nc.gpsimd.tensor_add(u2[:], u[:, 0, :, :], u[:, 1, :, :])
            u2s[(qb, kt)] = u2

        def norm_tail(qb, kt):
            e, u2 = es[(qb, kt)], u2s[(qb, kt)]
            u3 = work.tile([128, 2, QB], bf16, tag="u3", bufs=2,
                           name=f"u3_{qb}_{kt}")
            nc.vector.tensor_add(u3[:], u2[:, 0:2, :], u2[:, 2:4, :])
            zf = work.tile([128, QB], f32, tag="zf", bufs=2,
                           name=f"zf{qb}_{kt}")
            nc.vector.tensor_add(zf[:], u3[:, 0, :], u3[:, 1, :])
            rf = work.tile([128, QB], f32, tag="rf", bufs=2,
                           name=f"rf{qb}_{kt}")
            nc.vector.reciprocal_approx_fast(rf[:], zf[:])
            rb = work.tile([128, QB], bf16, tag="rb", bufs=2,
                           name=f"rb{qb}_{kt}")
            nc.vector.tensor_copy(rb[:], rf[:])
            nc.vector.tensor_mul(
                e[:, 0:8, :], e[:, 0:8, :],
                rb[:].unsqueeze(1).broadcast_to([128, 8, QB]))
            nc.vector.tensor_mul(
                e[:, 8:16, :], e[:, 8:16, :],
                rb[:].unsqueeze(1).broadcast_to([128, 8, QB]))

        def av_half(qb, kt, half):
            # o[q,e] accumulation: lhsT = a_h[k, 128q], rhs = v_h[k, 64e];
            # half selects the q-half (out psum qh)
            e = es[(qb, kt)]
            qh = half
            if kt == 0:
                o_ps_all[(qb, qh)] = psum_o.tile(
                    [128, 1024], f32, tag=f"o{qh}", name=f"o{qb}_{qh}")
            ops = o_ps_all[(qb, qh)]
            for h in range(H):
                nc.tensor.matmul(ops[:, h * 64:(h + 1) * 64],
                                 e[:, h, qh * 128:(qh + 1) * 128],
                                 v[:, kt, h, :],
                                 start=(kt == 0 and h % 8 == 0),
                                 stop=(kt == NKT - 1 and h % 8 == 7),
                                 skip_group_check=True)

        o_sbs = {}
        oTs = {}

        def p3_evac(qb, qh):
            o_sb = work.tile([128, 1024], bf16, tag="osb", bufs=2,
                             name=f"osb{qb}_{qh}")
            if qh == 0:
                nc.scalar.activation(o_sb[:], o_ps_all[(qb, qh)][:], AF.Copy)
            else:
                nc.vector.tensor_copy(o_sb[:], o_ps_all[(qb, qh)][:])
            o_sbs[(qb, qh)] = o_sb

        def p3_transpose(qb, qh):
            o_sb = o_sbs[(qb, qh)]
            tp = psum_s.tile([128, 1024], bf16, tag="s", name=f"tp{qb}_{qh}")
            for c in range(8):
                nc.tensor.matmul(tp[:, c * 128:(c + 1) * 128],
                                 o_sb[:, c * 128:(c + 1) * 128], id_t[:],
                                 is_transpose=True,
                                 start=(c % 4 == 0), stop=(c % 4 == 3),
                                 skip_group_check=True)
            oT = work.tile([128, 8, 128], bf16, tag="oT", bufs=2,
                           name=f"oT{qb}_{qh}")
            nc.vector.tensor_copy(oT[:], tp.rearrange("p (c q) -> p c q", c=8))
            oTs[(qb, qh)] = oT

        def p3_yout(qb, qh):
            q0 = qb * QB
            oT = oTs[(qb, qh)]
            y_ps = psum_s.tile([128, 1024], f32, tag="s", name=f"y{qb}_{qh}")
            for fb in range(2):
                for c in range(8):
                    nc.tensor.matmul(y_ps[:, fb * 512:(fb + 1) * 512],
                                     oT[:, c, :],
                                     wio[:, c, fb * 512:(fb + 1) * 512],
                                     start=(c == 0), stop=False)
                nc.tensor.matmul(y_ps[:, fb * 512:(fb + 1) * 512], ones_t[:],
                                 boro_t[0:1, fb * 512:(fb + 1) * 512],
                                 start=False, stop=True)
            y_sb = work.tile([128, 1024], f32, tag="y", bufs=1,
                             name=f"ysb{qb}_{qh}")
            nc.vector.tensor_copy(y_sb[:], y_ps[:])
            nc.sync.dma_start(
                d["out"][q0 + qh * 128:q0 + (qh + 1) * 128, :], y_sb[:])

        def p3_piece(qb, kt):
            # kt==2: av of (qb,15) was just emitted (lag-3) -> safe to evac
            if kt == 2:
                p3_evac(qb, 0)
                p3_evac(qb, 1)
            elif kt == 4:
                p3_transpose(qb, 0)
            elif kt == 6:
                p3_yout(qb, 0)
            elif kt == 8:
                p3_transpose(qb, 1)
            elif kt == 10:
                p3_yout(qb, 1)

        tiles = [(qb, kt) for qb in range(NQB) for kt in range(NKT)]
        for i, (qb, kt) in enumerate(tiles):
            if qb == 0 and kt % 4 == 0:
                kproj(kt // 4)
                for kch in range(kt, kt + 4):
                    vproj(kch)
                if kt == 12:   # Ws no longer needed after the last vproj
                    nc.sync.dma_start(
                        wio[:], d["woT"].rearrange("(c p) e -> p c e", p=128))
            e = work.tile([128, H, QB], bf16, tag="e", bufs=4,
                          name=f"e{qb}_{kt}")
            es[(qb, kt)] = e
            scores_half(qb, kt, 0, e)
            if i >= 3:
                av_half(*tiles[i - 3], 0)
            us[(qb, kt)] = tree_l1a(qb, kt)
            scores_half(qb, kt, 1, e)
            if i >= 3:
                av_half(*tiles[i - 3], 1)
            tree_l1b(qb, kt)
            if i >= 1:
                l2_piece(*tiles[i - 1])
            if i >= 2:
                norm_tail(*tiles[i - 2])
            if qb > 0:
                p3_piece(qb - 1, kt)
        n = len(tiles)
        l2_piece(*tiles[n - 1])
        norm_tail(*tiles[n - 1 - 1])
        norm_tail(*tiles[n - 1])
        for j in (n - 3, n - 2, n - 1):
            av_half(*tiles[j], 0)
            av_half(*tiles[j], 1)
        for kt in range(NKT):
            p3_piece(NQB - 1, kt)


def _get_nc():
    if "nc" not in _CACHE:
        _CACHE["nc"] = _build_bass()
    return _CACHE["nc"]


def _make_in_maps(query, key, value, W_split, b_split, W_o, b_o):
    import ml_dtypes
    bf16 = ml_dtypes.bfloat16

    query = np.asarray(query, np.float32)
    key = np.asarray(key, np.float32)
    value = np.asarray(value, np.float32)
    W_split = np.asarray(W_split, np.float32)
    b_split = np.asarray(b_split, np.float32)
    W_o = np.asarray(W_o, np.float32)
    b_o = np.asarray(b_o, np.float32)

    wsT = np.ascontiguousarray(W_split.T).astype(bf16)
    woT = np.ascontiguousarray(W_o.T).astype(bf16)
    bq = np.ascontiguousarray((b_split / 8.0).reshape(8, 128).T)
    bk = np.ascontiguousarray(b_split.reshape(8, 128).T)
    bvro = np.ascontiguousarray(b_split.reshape(1, D)).astype(bf16)
    boro = np.ascontiguousarray(b_o.reshape(1, D)).astype(bf16)
    ident = np.eye(128, dtype=bf16)

    kTs = [np.ascontiguousarray(key[:, b, :].T).astype(bf16)
           for b in range(BATCH)]
    vTs = [np.ascontiguousarray(value[:, b, :].T).astype(bf16)
           for b in range(BATCH)]
    in_maps = []
    for dcore in range(NCORES):
        b, qc = dcore // 2, dcore % 2
        xqT = np.ascontiguousarray(
            query[qc * QCH:(qc + 1) * QCH, b, :].T).astype(bf16)
        in_maps.append({
            "xqT": xqT, "xkT": kTs[b], "xvT": vTs[b],
            "wsT": wsT, "woT": woT,
            "bq": bq, "bk": bk, "bvro": bvro, "boro": boro,
            "ident": ident,
        })
    return in_maps


def kernel_with_results(trace=False, **inputs):
    from concourse.bass_utils import run_bass_kernel_spmd

    nc = _get_nc()
    in_maps = _make_in_maps(**inputs)
    last_exc = None
    for _attempt in range(3):
        try:
            res = run_bass_kernel_spmd(nc, in_maps,
                                       core_ids=list(range(NCORES)),
                                       trace=trace)
            break
        except Exception as exc:  # rare transient device fault -> retry
            last_exc = exc
    else:
        raise last_exc
    out = np.empty((SEQ, BATCH, D), np.float32)
    for dcore in range(NCORES):
        b, qc = dcore // 2, dcore % 2
        out[qc * QCH:(qc + 1) * QCH, b, :] = res.results[dcore]["out"]
    return out, res


def kernel(**inputs):
    out, _ = kernel_with_results(trace=False, **inputs)
    return out


# revision 23
# speedup vs baseline: 1.1368x; 1.1368x over previous
"""Trainium2 Bass kernel for nn_MultiHeadAttention_87239375716860.

Softmax-over-HEADS MHA (reference quirk). Sharding: 8 cores = 4 batches
x 2 query-halves, no collectives.

v2 changes vs baseline:
  - all matmul operands bf16 (host converts); input DMA halved
  - attn@V in o[q,e] form: lhsT = a_h[k,128q], rhs = v[k,64e] -> full
    128-contraction PE efficiency (halves that phase's PE columns)
  - out-projection via PE transposes of o[q,e] (identity input tensor)
  - softmax engine rebalance: exp on Act, tree l1/l3/l4+recip+muls on
    DVE, l2 on Pool, v/y biases on Pool
  - k/v projections pipelined inside qb0's attention sweep
PSUM budget: psum_s tag "s" [128,1024] x2 bufs (4 banks) shared by
scores / transposes / y; psum_o tags o0,o1 [128,1024] x1 buf (4 banks)
for the two q-half o accumulators.  start/stop are BANK-granular.
"""

import numpy as np

SEQ = 2048
BATCH = 4
D = 1024
H = 16
DK = 64
QCH = 1024          # queries per core
NCORES = 8
QB = 256            # q-block for scores/softmax
NQB = QCH // QB     # 4
NKT = SEQ // 128    # 16 k-tiles

_CACHE = {}


def _build_bass(n_iters=1):
    import concourse.tile as tile
    from concourse import bacc, mybir

    f32 = mybir.dt.float32
    bf16 = mybir.dt.bfloat16

    nc = bacc.Bacc("TRN2", target_bir_lowering=False, debug=False,
                   num_devices=NCORES)

    d = {}
    d["xqT"] = nc.dram_tensor("xqT", [D, QCH], bf16, kind="ExternalInput").ap()
    d["xkT"] = nc.dram_tensor("xkT", [D, SEQ], bf16, kind="ExternalInput").ap()
    d["xvT"] = nc.dram_tensor("xvT", [D, SEQ], bf16, kind="ExternalInput").ap()
    d["wsT"] = nc.dram_tensor("wsT", [D, D], bf16, kind="ExternalInput").ap()
    d["woT"] = nc.dram_tensor("woT", [D, D], bf16, kind="ExternalInput").ap()
    d["bq"] = nc.dram_tensor("bq", [128, 8], f32, kind="ExternalInput").ap()
    d["bk"] = nc.dram_tensor("bk", [128, 8], f32, kind="ExternalInput").ap()
    d["bvro"] = nc.dram_tensor("bvro", [1, D], bf16,
                               kind="ExternalInput").ap()
    d["boro"] = nc.dram_tensor("boro", [1, D], bf16,
                               kind="ExternalInput").ap()
    d["ident"] = nc.dram_tensor("ident", [128, 128], bf16,
                                kind="ExternalInput").ap()
    d["out"] = nc.dram_tensor("out", [QCH, D], f32, kind="ExternalOutput").ap()

    with tile.TileContext(nc) as tc:
        for _it in range(n_iters):
            _emit_iter(nc, tc, mybir, tile, d)

    nc.compile()
    return nc


def _emit_iter(nc, tc, mybir, tile, d):
    from contextlib import ExitStack

    f32 = mybir.dt.float32
    bf16 = mybir.dt.bfloat16
    AF = mybir.ActivationFunctionType

    with ExitStack() as ctx:
        persist = ctx.enter_context(tc.tile_pool(name="persist", bufs=1))
        psum_s = ctx.enter_context(
            tc.tile_pool(name="psum_s", space="PSUM", bufs=2))
        psum_o = ctx.enter_context(
            tc.tile_pool(name="psum_o", space="PSUM", bufs=1))
        work = ctx.enter_context(tc.tile_pool(name="work", bufs=1))

        # wio holds Ws^T during projections, then Wo^T (loaded after the
        # last vproj) for the out-projection.
        wio = persist.tile([128, 8, D], bf16, name="wio")
        qT2 = persist.tile([128, 8, 2, QCH], bf16, name="qT2")
        kT = persist.tile([128, 8, SEQ], bf16, name="kT")
        v = persist.tile([128, NKT, H, DK], bf16, name="v")
        id_t = persist.tile([128, 128], bf16, name="id_t")
        bq_t = persist.tile([128, 8], f32, name="bq_t")
        bk_t = persist.tile([128, 8], f32, name="bk_t")
        bvro_t = persist.tile([1, D], bf16, name="bvro_t")
        boro_t = persist.tile([1, D], bf16, name="boro_t")
        ones_t = persist.tile([1, 128], bf16, name="ones_t")

        nc.sync.dma_start(wio[:], d["wsT"].rearrange("(c p) e -> p c e",
                                                     p=128))
        nc.gpsimd.memset(qT2[64:128, :, 0, :], 0.0)
        nc.gpsimd.memset(qT2[0:64, :, 1, :], 0.0)
        nc.sync.dma_start(id_t[:], d["ident"])
        nc.sync.dma_start(bq_t[:], d["bq"])
        nc.sync.dma_start(bk_t[:], d["bk"])
        nc.sync.dma_start(bvro_t[:], d["bvro"])
        nc.sync.dma_start(boro_t[:], d["boro"])
        nc.gpsimd.memset(ones_t[:], 1.0)

        # ---- qT projection (zero-padded parity trick) ----
        for sb in range(QCH // 512):
            xq_s = work.tile([128, 8, 512], bf16, tag="xs", bufs=2,
                             name=f"xq{sb}")
            nc.sync.dma_start(
                xq_s[:], d["xqT"][:, sb * 512:(sb + 1) * 512]
                .rearrange("(c p) s -> p c s", p=128))
            for c in range(8):
                ps = psum_s.tile([128, 1024], f32, tag="s", name=f"psq{sb}{c}")
                for dch in range(8):
                    nc.tensor.matmul(ps[:, 0:512],
                                     wio[:, dch, c * 128:(c + 1) * 128],
                                     xq_s[:, dch, :],
                                     start=(dch == 0), stop=(dch == 7))
                nc.scalar.activation(
                    qT2[0:64, c, 0, sb * 512:(sb + 1) * 512], ps[0:64, 0:512],
                    AF.Identity, bias=bq_t[0:64, c:c + 1], scale=0.125)
                nc.scalar.activation(
                    qT2[64:128, c, 1, sb * 512:(sb + 1) * 512],
                    ps[64:128, 0:512],
                    AF.Identity, bias=bq_t[64:128, c:c + 1], scale=0.125)

        def kproj(sb):
            xk_s = work.tile([128, 8, 512], bf16, tag="xs", bufs=2,
                             name=f"xk{sb}")
            nc.sync.dma_start(
                xk_s[:], d["xkT"][:, sb * 512:(sb + 1) * 512]
                .rearrange("(c p) s -> p c s", p=128))
            for c in range(8):
                ps = psum_s.tile([128, 1024], f32, tag="s", name=f"psk{sb}{c}")
                for dch in range(8):
                    nc.tensor.matmul(ps[:, 0:512],
                                     wio[:, dch, c * 128:(c + 1) * 128],
                                     xk_s[:, dch, :],
                                     start=(dch == 0), stop=(dch == 7))
                nc.scalar.activation(kT[:, c, sb * 512:(sb + 1) * 512],
                                     ps[:, 0:512], AF.Identity,
                                     bias=bk_t[:, c:c + 1])

        def vproj(kch):
            xv_s = work.tile([128, 8, 128], bf16, tag="xv", bufs=2,
                             name=f"xv{kch}")
            nc.sync.dma_start(
                xv_s[:], d["xvT"][:, kch * 128:(kch + 1) * 128]
                .rearrange("(c p) k -> p c k", p=128))
            ps = psum_s.tile([128, 1024], f32, tag="s", name=f"psv{kch}")
            for eb in range(2):
                for dch in range(8):
                    nc.tensor.matmul(ps[:, eb * 512:(eb + 1) * 512],
                                     xv_s[:, dch, :],
                                     wio[:, dch, eb * 512:(eb + 1) * 512],
                                     start=(dch == 0), stop=False)
                # bias: ones^T (x) b_split row, contraction-1 accumulate
                nc.tensor.matmul(ps[:, eb * 512:(eb + 1) * 512], ones_t[:],
                                 bvro_t[0:1, eb * 512:(eb + 1) * 512],
                                 start=False, stop=True)
            pv = ps.rearrange("p (h k) -> p h k", h=H)
            nc.scalar.activation(v[:, kch, :, :], pv[:], AF.Copy)

        # ---- software-pipelined attention ----
        # stage layout per emission step (qb, kt):
        #   PE : scores(kt) hg0,hg1 | av(kt-2) h0-7 | scores hg2,hg3 |
        #        av(kt-2) h8-15
        #   Act: exp(kt) hg0..3
        #   Pool: l1(kt)  (after exp hg3)
        #   DVE: l2..muls of (kt-1)
        es = {}          # (qb,kt) -> e tile
        o_ps_all = {}

        def scores_half(qb, kt, half, e):
            q0 = qb * QB
            for hg in (0, 1) if half == 0 else (2, 3):
                ps = psum_s.tile([128, 1024], f32, tag="s",
                                 name=f"pss{qb}_{kt}_{hg}")
                for cl in range(2):
                    c = hg * 2 + cl
                    nc.tensor.matmul(ps[:, cl * 512:(cl + 1) * 512],
                                     kT[:, c, kt * 128:(kt + 1) * 128],
                                     qT2[:, c, :, q0:q0 + QB],
                                     start=True, stop=True)
                nc.scalar.activation(e[:, hg * 4:(hg + 1) * 4, :], ps[:, :],
                                     AF.Exp)

        def tree_l1a(qb, kt):
            e = es[(qb, kt)]
            u = work.tile([128, 2, 4, QB], bf16, tag="u", bufs=2,
                          name=f"u{qb}_{kt}")
            nc.vector.tensor_add(u[:, 0, :, :], e[:, 0:4, :], e[:, 4:8, :])
            return u

        def tree_l1b(qb, kt):
            e, u = es[(qb, kt)], us[(qb, kt)]
            nc.vector.tensor_add(u[:, 1, :, :], e[:, 8:12, :], e[:, 12:16, :])

        us = {}

        u2s = {}

        def l2_piece(qb, kt):
            u = us[(qb, kt)]
            u2 = work.tile([128, 4, QB], bf16, tag="u2", bufs=2,
                           name=f"u2_{qb}_{kt}")
            nc.gpsimd.tensor_add(u2[:], u[:, 0, :, :], u[:, 1, :, :])
            u2s[(qb, kt)] = u2

        def norm_tail(qb, kt):
            e, u2 = es[(qb, kt)], u2s[(qb, kt)]
            u3 = work.tile([128, 2, QB], bf16, tag="u3", bufs=2,
                           name=f"u3_{qb}_{kt}")
            nc.vector.tensor_add(u3[:], u2[:, 0:2, :], u2[:, 2:4, :])
            zf = work.tile([128, QB], f32, tag="zf", bufs=2,
                           name=f"zf{qb}_{kt}")
            nc.vector.tensor_add(zf[:], u3[:, 0, :], u3[:, 1, :])
            rf = work.tile([128, QB], f32, tag="rf", bufs=2,
                           name=f"rf{qb}_{kt}")
            nc.vector.reciprocal_approx_fast(rf[:], zf[:])
            rb = work.tile([128, QB], bf16, tag="rb", bufs=2,
                           name=f"rb{qb}_{kt}")
            nc.vector.tensor_copy(rb[:], rf[:])
            nc.vector.tensor_mul(
                e[:, 0:8, :], e[:, 0:8, :],
                rb[:].unsqueeze(1).broadcast_to([128, 8, QB]))
            nc.vector.tensor_mul(
                e[:, 8:16, :], e[:, 8:16, :],
                rb[:].unsqueeze(1).broadcast_to([128, 8, QB]))

        def av_half(qb, kt, half):
            # o[q,e] accumulation: lhsT = a_h[k, 128q], rhs = v_h[k, 64e];
            # half selects the q-half (out psum qh)
            e = es[(qb, kt)]
            qh = half
            if kt == 0:
                o_ps_all[(qb, qh)] = psum_o.tile(
                    [128, 1024], f32, tag=f"o{qh}", name=f"o{qb}_{qh}")
            ops = o_ps_all[(qb, qh)]
            for h in range(H):
                nc.tensor.matmul(ops[:, h * 64:(h + 1) * 64],
                                 e[:, h, qh * 128:(qh + 1) * 128],
                                 v[:, kt, h, :],
                                 start=(kt == 0 and h % 8 == 0),
                                 stop=(kt == NKT - 1 and h % 8 == 7),
                                 skip_group_check=True)

        o_sbs = {}
        oTs = {}

        def p3_evac(qb, qh):
            o_sb = work.tile([128, 1024], bf16, tag="osb", bufs=2,
                             name=f"osb{qb}_{qh}")
            if qh == 0:
                nc.scalar.activation(o_sb[:], o_ps_all[(qb, qh)][:], AF.Copy)
            else:
                nc.vector.tensor_copy(o_sb[:], o_ps_all[(qb, qh)][:])
            o_sbs[(qb, qh)] = o_sb

        def p3_transpose(qb, qh):
            o_sb = o_sbs[(qb, qh)]
            tp = psum_s.tile([128, 1024], bf16, tag="s", name=f"tp{qb}_{qh}")
            for c in range(8):
                nc.tensor.matmul(tp[:, c * 128:(c + 1) * 128],
                                 o_sb[:, c * 128:(c + 1) * 128], id_t[:],
                                 is_transpose=True,
                                 start=(c % 4 == 0), stop=(c % 4 == 3),
                                 skip_group_check=True)
            oT = work.tile([128, 8, 128], bf16, tag="oT", bufs=2,
                           name=f"oT{qb}_{qh}")
            nc.vector.tensor_copy(oT[:], tp.rearrange("p (c q) -> p c q", c=8))
            oTs[(qb, qh)] = oT

        def p3_yout(qb, qh):
            q0 = qb * QB
            oT = oTs[(qb, qh)]
            y_ps = psum_s.tile([128, 1024], f32, tag="s", name=f"y{qb}_{qh}")
            for fb in range(2):
                for c in range(8):
                    nc.tensor.matmul(y_ps[:, fb * 512:(fb + 1) * 512],
                                     oT[:, c, :],
                                     wio[:, c, fb * 512:(fb + 1) * 512],
                                     start=(c == 0), stop=False)
                nc.tensor.matmul(y_ps[:, fb * 512:(fb + 1) * 512], ones_t[:],
                                 boro_t[0:1, fb * 512:(fb + 1) * 512],
                                 start=False, stop=True)
            y_sb = work.tile([128, 1024], f32, tag="y", bufs=1,
                             name=f"ysb{qb}_{qh}")
            nc.vector.tensor_copy(y_sb[:], y_ps[:])
            nc.sync.dma_start(
                d["out"][q0 + qh * 128:q0 + (qh + 1) * 128, :], y_sb[:])

        def p3_piece(qb, kt):
            # kt==2: av of (qb,15) was just emitted (lag-3) -> safe to evac
            if kt == 2:
                p3_evac(qb, 0)
                p3_evac(qb, 1)
            elif kt == 4:
                p3_transpose(qb, 0)
            elif kt == 6:
                p3_yout(qb, 0)
            elif kt == 8:
                p3_transpose(qb, 1)
            elif kt == 10:
                p3_yout(qb, 1)

        tiles = [(qb, kt) for qb in range(NQB) for kt in range(NKT)]
        for i, (qb, kt) in enumerate(tiles):
            if qb == 0 and kt % 4 == 0:
                kproj(kt // 4)
                for kch in range(kt, kt + 4):
                    vproj(kch)
                if kt == 12:   # Ws no longer needed after the last vproj
                    nc.sync.dma_start(
                        wio[:], d["woT"].rearrange("(c p) e -> p c e", p=128))
            e = work.tile([128, H, QB], bf16, tag="e", bufs=4,
                          name=f"e{qb}_{kt}")
            es[(qb, kt)] = e
            scores_half(qb, kt, 0, e)
            if i >= 3:
                av_half(*tiles[i - 3], 0)
            us[(qb, kt)] = tree_l1a(qb, kt)
            scores_half(qb, kt, 1, e)
            if i >= 3:
                av_half(*tiles[i - 3], 1)
            tree_l1b(qb, kt)
            if i >= 1:
                l2_piece(*tiles[i - 1])
            if i >= 2:
                norm_tail(*tiles[i - 2])
            if qb > 0:
                p3_piece(qb - 1, kt)
        n = len(tiles)
        l2_piece(*tiles[n - 1])
        norm_tail(*tiles[n - 1 - 1])
        norm_tail(*tiles[n - 1])
        for j in (n - 3, n - 2, n - 1):
            av_half(*tiles[j], 0)
            av_half(*tiles[j], 1)
        for kt in range(NKT):
            p3_piece(NQB - 1, kt)


def _get_nc():
    if "nc" not in _CACHE:
        _CACHE["nc"] = _build_bass()
    return _CACHE["nc"]


def _make_in_maps(query, key, value, W_split, b_split, W_o, b_o):
    import ml_dtypes
    bf16 = ml_dtypes.bfloat16

    query = np.asarray(query, np.float32)
    key = np.asarray(key, np.float32)
    value = np.asarray(value, np.float32)
    W_split = np.asarray(W_split, np.float32)
    b_split = np.asarray(b_split, np.float32)
    W_o = np.asarray(W_o, np.float32)
    b_o = np.asarray(b_o, np.float32)

    wsT = np.ascontiguousarray(W_split.T).astype(bf16)
    woT = np.ascontiguousarray(W_o.T).astype(bf16)
    bq = np.ascontiguousarray((b_split / 8.0).reshape(8, 128).T)
    bk = np.ascontiguousarray(b_split.reshape(8, 128).T)
    bvro = np.ascontiguousarray(b_split.reshape(1, D)).astype(bf16)
    boro = np.ascontiguousarray(b_o.reshape(1, D)).astype(bf16)
    ident = np.eye(128, dtype=bf16)

    kTs = [np.ascontiguousarray(key[:, b, :].T).astype(bf16)
           for b in range(BATCH)]
    vTs = [np.ascontiguousarray(value[:, b, :].T).astype(bf16)
           for b in range(BATCH)]
    in_maps = []
    for dcore in range(NCORES):
        b, qc = dcore // 2, dcore % 2
        xqT = np.ascontiguousarray(
            query[qc * QCH:(qc + 1) * QCH, b, :].T).astype(bf16)
        in_maps.append({
            "xqT": xqT, "xkT": kTs[b], "xvT": vTs[b],
            "wsT": wsT, "woT": woT,
            "bq": bq, "bk": bk, "bvro": bvro, "boro": boro,
            "ident": ident,
        })
    return in_maps


def kernel_with_results(trace=False, **inputs):
    from concourse.bass_utils import run_bass_kernel_spmd

    nc = _get_nc()
    in_maps = _make_in_maps(**inputs)
    last_exc = None
    for _attempt in range(3):
        try:
            res = run_bass_kernel_spmd(nc, in_maps,
                                       core_ids=list(range(NCORES)),
                                       trace=trace)
            break
        except Exception as exc:  # rare transient device fault -> retry
            last_exc = exc
    else:
        raise last_exc
    out = np.empty((SEQ, BATCH, D), np.float32)
    for dcore in range(NCORES):
        b, qc = dcore // 2, dcore % 2
        out[qc * QCH:(qc + 1) * QCH, b, :] = res.results[dcore]["out"]
    return out, res


def kernel(**inputs):
    out, _ = kernel_with_results(trace=False, **inputs)
    return out
